# revision 8
# baseline (speedup 1.0000x reference)
"""Trainium2 Bass kernel for DigitConvolutionalModel.

Model: x[B,784] -> reshape 28x28 -> 3x3 valid conv (weights conv_w) ->
[B,676] -> Linear(676,100)+relu -> Linear(100,10)+relu -> Linear(10,10).

The conv is linear, so it folds into the first Linear: W1f = C @ w1 where
C[784,676] is the conv unfold matrix. The whole model becomes a 3-layer MLP
784 -> 100 -> 10 -> 10 with relu between layers.

Sharding: pure data parallel, batch split across 8 cores (8192 rows each).

Precision: matmuls in bf16, accumulation in fp32 PSUM, biases + output fp32.

On-chip layout: activations stay feature-major ([features, batch]) end to
end; weights in natural [in,out] layout are the stationary operand.

v2 structure (from trace analysis of v1 @ 63.9us):
- x loads are granule-sized ([1,1,2,2,2,2,2,2,2] supertiles per dma_start,
  12KB/partition for the pair granules) on the sync queue, every granule in
  its own SBUF buffer (96KB total, no reuse waits on the load queue). One
  DMA sem per granule: PE matmuls carrying sem waits cost +88ns each
  (SW-decode path), so fewer, bigger transfers cut the PE wait tax.
- weights ride the scalar (ACT) HWDGE queue: that engine finishes its
  preamble ~1.5us before sync, so the blob lands before the first x tile.
- per-pair [*, 2, TN] PSUM tiles spanning 2 banks: one ACT activation per
  layer per pair (instead of per supertile) halves ACT instructions and PE
  cross-engine waits. PSUM budget: L1 2x2 + L2 2 + L3 2 = 8 banks.
- b3 is applied per-partition by the DVE scalar_tensor_tensor (add, +0),
  dropping the broadcast b3 image from the blob (447KB -> 186KB).
- short warmup (9 x 256-col dummy matmuls) bridges PE boot (~6.8us) to
  first-data (~10us) so the HAM clock is warm when real matmuls start.
- last pair processed as two split supertile chains (ACT half / DVE half in
  parallel) to shorten the serial L1->L2->L3->store tail.
"""

import numpy as np
import ml_dtypes

import concourse.bacc as bacc
import concourse.tile as tile
from concourse.tile import add_dep_helper
from concourse import mybir
from concourse.bass_utils import run_bass_kernel_spmd

N_CORES = 8
B = 65536
BC = B // N_CORES  # 8192 rows per core
TN = 512           # batch columns per supertile
NT = BC // TN      # 16 supertiles per core
NKC = 6            # full 128-feature chunks (0..767)
KT = 16            # tail features (768..783)
NF = 784
H1 = 100
HO = 10
F32 = mybir.dt.float32
BF16 = mybir.dt.bfloat16
NP_BF16 = ml_dtypes.bfloat16

# loads: (supertile, first chunk, n chunks). Chunk-split small loads at the
# edges (fast first-data at the front, short receipt exposure at the back),
# 2-supertile loads in the middle. Multi-supertile loads have k0=0, nk=6*s.
LOADS = [
    (0, 0, 3), (0, 3, 3), (1, 0, 3), (1, 3, 3),
    (2, 0, 12), (4, 0, 12), (6, 0, 12), (8, 0, 12), (10, 0, 12),
    (12, 0, 12),
    (14, 0, 6), (15, 0, 3), (15, 3, 3),
]
NPAIR = NT // 2

# packed weight blob column layout (bf16 columns)
_C_W1M = 0                      # [128, 600]  w1m chunks
_C_W1T = 600                    # [16, 100]   w1t
_C_W2 = 700                     # [100, 10]   w2
_C_W3 = 710                     # [10, 10]    w3
_C_B1 = 720                     # [100, 2]    b1 as f32 byte-pairs
_C_B2 = 722                     # [10, 2]     b2
_C_B3 = 724                     # [10, 2]     b3
WBW = 726

N_WARMUP = 12
WUN = 256  # warmup matmul free dim


def _build_nc():
    nc = bacc.Bacc(None, target_bir_lowering=False)

    # feature-major, partition-major-first so any run of supertiles is
    # contiguous per partition: xt[p, t, k, n]
    xt = nc.dram_tensor("xt", [128, NT, NKC, TN], BF16, kind="ExternalInput")
    xt_tail = nc.dram_tensor("xt_tail", [KT, NT, TN], BF16, kind="ExternalInput")
    wblob = nc.dram_tensor("wblob", [128, WBW], BF16, kind="ExternalInput")
    yt = nc.dram_tensor("yt", [HO, BC], F32, kind="ExternalOutput")

    relu = mybir.ActivationFunctionType.Relu
    copy_fn = mybir.ActivationFunctionType.Identity

    with tile.TileContext(nc) as tc:
        with (
            tc.tile_pool(name="const", bufs=1) as cpool,
            tc.tile_pool(name="io", bufs=1) as iopool,
            tc.tile_pool(name="act", bufs=3) as apool,
            tc.tile_pool(name="ps1", bufs=2, space="PSUM") as ps1,
            tc.tile_pool(name="ps2", bufs=1, space="PSUM") as ps2,
            tc.tile_pool(name="ps3", bufs=1, space="PSUM") as ps3,
        ):
            # weights on the scalar HWDGE queue (boots earliest)
            wb_s = cpool.tile([128, WBW], BF16, tag="wb")
            nc.scalar.dma_start(wb_s[:], wblob[:])
            # tails on the gpsimd (SWDGE) queue
            xtl_s = cpool.tile([KT, NT, TN], BF16, tag="xtl")
            nc.gpsimd.dma_start(xtl_s[:], xt_tail[:])
            # loads, all issued upfront on the sync queue, each into its own
            # buffer (no reuse waits on the load queue)
            chunk_ap = {}
            for li, (t0, k0, nk) in enumerate(LOADS):
                t_ = iopool.tile([128, nk, TN], BF16, tag=f"x{li}")
                if nk > NKC:
                    s = nk // NKC
                    nc.sync.dma_start(t_[:], xt[:, t0:t0 + s])
                    for j in range(s):
                        for k in range(NKC):
                            chunk_ap[(t0 + j, k)] = t_[:, j * NKC + k, :]
                else:
                    nc.sync.dma_start(t_[:], xt[:, t0, k0:k0 + nk])
                    for k in range(nk):
                        chunk_ap[(t0, k0 + k)] = t_[:, k, :]

            w1t_ap = wb_s[0:KT, _C_W1T:_C_W1T + H1]
            w2_ap = wb_s[0:H1, _C_W2:_C_W2 + HO]
            w3_ap = wb_s[0:HO, _C_W3:_C_W3 + HO]
            b1_ap = wb_s[0:H1, _C_B1:_C_B1 + 2].bitcast(F32)
            b2_ap = wb_s[0:HO, _C_B2:_C_B2 + 2].bitcast(F32)
            b3_ap = wb_s[0:HO, _C_B3:_C_B3 + 2].bitcast(F32)

            # All matmuls chained with same-engine ordering deps so the PE
            # executes them in emission order (required for ldweights=False
            # weight reuse from the previous matmul).
            prev_mm = [None]

            def mm(out_ap, lhsT_ap, rhs_ap, start, stop, ldw=True):
                m = nc.tensor.matmul(out_ap, lhsT_ap, rhs_ap,
                                     start=start, stop=stop)
                if not ldw:
                    m.ins.ldweights = False
                if prev_mm[0] is not None:
                    add_dep_helper(m.ins, prev_mm[0], sync=False,
                                   reason="pe program order")
                prev_mm[0] = m.ins
                return m

            # Warmup: dummy matmuls bridge the PE-boot -> first-data window
            # so the HAM clock is warm for the real stream. They multiply
            # garbage (wsc is memset AFTER emission: WAR, not RAW, so they
            # start at the engines-go barrier).
            wsc = cpool.tile([128, 2 * TN], BF16, tag="wsc")
            wp = ps1.tile([H1, 2, TN], F32, tag="p1")
            mm(wp[:, 0, 0:WUN], wsc[:, 0:H1], wsc[:, 0:WUN],
               start=True, stop=True)
            for _ in range(N_WARMUP - 1):
                mm(wp[:, 0, 0:WUN], wsc[:, 0:H1], wsc[:, 0:WUN],
                   start=True, stop=True, ldw=False)
            nc.vector.memset(wsc[:], 0.0)

            def xap(t, k):
                return chunk_ap[(t, k)]

            h1s: dict[int, object] = {}
            h2s: dict[int, object] = {}

            def emit_l1(p):
                a, b = 2 * p, 2 * p + 1
                last = p == NPAIR - 1
                p1 = ps1.tile([H1, 2, TN], F32, tag="p1")
                if p == 0 or last:
                    # edge pairs use chunk-split loads: process each
                    # supertile sequentially, k ascending, aligned with
                    # load arrival order.
                    for j, t in ((0, a), (1, b)):
                        for k in range(NKC):
                            mm(p1[:, j, :], wb_s[:, k * H1:(k + 1) * H1],
                               xap(t, k), start=(k == 0), stop=False)
                        mm(p1[:, j, :], w1t_ap, xtl_s[:, t, :],
                           start=False, stop=True, ldw=(j == 0))
                else:
                    for k in range(NKC):
                        mm(p1[:, 0, :], wb_s[:, k * H1:(k + 1) * H1],
                           xap(a, k), start=(k == 0), stop=False)
                        mm(p1[:, 1, :], wb_s[:, k * H1:(k + 1) * H1],
                           xap(b, k), start=(k == 0), stop=False, ldw=False)
                    mm(p1[:, 0, :], w1t_ap, xtl_s[:, a, :],
                       start=False, stop=True)
                    mm(p1[:, 1, :], w1t_ap, xtl_s[:, b, :],
                       start=False, stop=True, ldw=False)
                h1 = apool.tile([H1, 2, TN], BF16, tag="h1")
                if last:
                    # last pair: two parallel half-chains (ACT / DVE) to
                    # shorten the serial drain after the final L1 matmul
                    nc.scalar.activation(h1[:, 0, :], p1[:, 0, :], relu,
                                         bias=b1_ap)
                    nc.vector.scalar_tensor_tensor(
                        h1[:, 1, :], p1[:, 1, :], b1_ap, wsc[0:H1, 0:TN],
                        op0=mybir.AluOpType.add, op1=mybir.AluOpType.max)
                else:
                    nc.scalar.activation(h1[:], p1[:], relu, bias=b1_ap)
                h1s[p] = h1

            def emit_l2(p):
                h1 = h1s.pop(p)
                p2 = ps2.tile([HO, 2, TN], F32, tag="p2")
                mm(p2[:, 0, :], w2_ap, h1[:, 0, :], start=True, stop=True)
                mm(p2[:, 1, :], w2_ap, h1[:, 1, :], start=True, stop=True,
                   ldw=False)
                h2 = apool.tile([HO, 2, TN], BF16, tag="h2")
                if p == NPAIR - 1:
                    nc.scalar.activation(h2[:, 0, :], p2[:, 0, :], relu,
                                         bias=b2_ap)
                    nc.vector.scalar_tensor_tensor(
                        h2[:, 1, :], p2[:, 1, :], b2_ap, wsc[0:HO, 0:TN],
                        op0=mybir.AluOpType.add, op1=mybir.AluOpType.max)
                else:
                    nc.scalar.activation(h2[:], p2[:], relu, bias=b2_ap)
                h2s[p] = h2

            def emit_l3(p):
                h2 = h2s.pop(p)
                p3 = ps3.tile([HO, 2, TN], F32, tag="p3")
                mm(p3[:, 0, :], w3_ap, h2[:, 0, :], start=True, stop=True)
                mm(p3[:, 1, :], w3_ap, h2[:, 1, :], start=True, stop=True,
                   ldw=False)
                ot = apool.tile([HO, 2, TN], F32, tag="ot")
                dst = yt[:, 2 * p * TN:(2 * p + 2) * TN]
                if p == NPAIR - 1:
                    # split halves: a finishes on ACT, b on DVE, stores
                    # issue as each half lands
                    nc.scalar.activation(ot[:, 0, :], p3[:, 0, :], copy_fn,
                                         bias=b3_ap)
                    nc.sync.dma_start(yt[:, 2 * p * TN:(2 * p + 1) * TN],
                                      ot[:, 0, :])
                    nc.vector.scalar_tensor_tensor(
                        ot[:, 1, :], p3[:, 1, :], b3_ap, wsc[0:HO, 0:TN],
                        op0=mybir.AluOpType.add, op1=mybir.AluOpType.add)
                    nc.sync.dma_start(yt[:, (2 * p + 1) * TN:(2 * p + 2) * TN],
                                      ot[:, 1, :])
                else:
                    # bias-add via ACT Copy keeps the DVE queue empty for
                    # the endgame half-chains
                    nc.scalar.activation(ot[:], p3[:], copy_fn, bias=b3_ap)
                    nc.sync.dma_start(dst, ot[:])

            # 3-stage software pipeline: L1(p), L2(p-1), L3(p-2)
            for p in range(NPAIR + 2):
                if p < NPAIR:
                    emit_l1(p)
                if 1 <= p <= NPAIR:
                    emit_l2(p - 1)
                if p >= 2:
                    emit_l3(p - 2)

    nc.compile()
    return nc


def _fold_conv_into_w1(conv_w: np.ndarray, w1: np.ndarray) -> np.ndarray:
    """W1f[784,100] such that x @ W1f == conv(x).reshape(B,676) @ w1."""
    c = np.zeros((NF, 26 * 26), dtype=np.float64)
    for di in range(3):
        for dj in range(3):
            ii, jj = np.meshgrid(np.arange(26), np.arange(26), indexing="ij")
            src = (ii + di) * 28 + (jj + dj)
            dst = ii * 26 + jj
            c[src.ravel(), dst.ravel()] += np.float64(conv_w[di, dj])
    return (c @ w1.astype(np.float64)).astype(np.float32)


def _prep_in_maps(x, conv_w, w1, b1, w2, b2, w3, b3):
    x = np.asarray(x, dtype=np.float32)
    conv_w = np.asarray(conv_w, dtype=np.float32)
    w1 = np.asarray(w1, dtype=np.float32)
    b1 = np.asarray(b1, dtype=np.float32)
    w2 = np.asarray(w2, dtype=np.float32)
    b2 = np.asarray(b2, dtype=np.float32)
    w3 = np.asarray(w3, dtype=np.float32)
    b3 = np.asarray(b3, dtype=np.float32)

    w1f = _fold_conv_into_w1(conv_w, w1)  # [784, 100]
    # main chunks: feature f = k*128 + p -> [128, 600]
    w1m = np.ascontiguousarray(
        w1f[: 128 * NKC].reshape(NKC, 128, H1).transpose(1, 0, 2)
    ).astype(NP_BF16).reshape(128, NKC * H1)
    w1t = w1f[128 * NKC:].astype(NP_BF16)  # [16, 100]

    blob = np.zeros((128, WBW), np.uint16)
    blob[:, _C_W1M:_C_W1M + NKC * H1] = w1m.view(np.uint16)
    blob[0:KT, _C_W1T:_C_W1T + H1] = w1t.view(np.uint16)
    blob[0:H1, _C_W2:_C_W2 + HO] = w2.astype(NP_BF16).view(np.uint16)
    blob[0:HO, _C_W3:_C_W3 + HO] = w3.astype(NP_BF16).view(np.uint16)
    blob[0:H1, _C_B1:_C_B1 + 2] = b1.reshape(H1, 1).view(np.uint16)
    blob[0:HO, _C_B2:_C_B2 + 2] = b2.reshape(HO, 1).view(np.uint16)
    blob[0:HO, _C_B3:_C_B3 + 2] = b3.reshape(HO, 1).view(np.uint16)
    shared = {"wblob": blob.view(NP_BF16)}

    xb = x.astype(NP_BF16)  # cast once, full batch
    in_maps = []
    for core in range(N_CORES):
        xc = xb[core * BC:(core + 1) * BC]  # [8192, 784] bf16
        xct = xc.reshape(NT, TN, NF).transpose(0, 2, 1)  # [NT, NF, TN]
        # [128, NT, NKC, TN]: partition-major so granules are contiguous
        xt_main = np.ascontiguousarray(
            xct[:, : 128 * NKC].reshape(NT, NKC, 128, TN).transpose(2, 0, 1, 3)
        )
        xt_tail = np.ascontiguousarray(
            xct[:, 128 * NKC:].transpose(1, 0, 2)
        )  # [KT, NT, TN]
        in_maps.append({"xt": xt_main, "xt_tail": xt_tail, **shared})
    return in_maps


_NC = None


def _get_nc():
    global _NC
    if _NC is None:
        _NC = _build_nc()
    return _NC


def kernel(x, conv_w, w1, b1, w2, b2, w3, b3):
    in_maps = _prep_in_maps(x, conv_w, w1, b1, w2, b2, w3, b3)
    nc = _get_nc()
    res = run_bass_kernel_spmd(nc, in_maps, core_ids=list(range(N_CORES)))
    out = np.empty((B, HO), dtype=np.float32)
    for i in range(N_CORES):
        out[i * BC:(i + 1) * BC] = res.results[i]["yt"].T
    return out


if __name__ == "__main__":
    rng = np.random.default_rng(0)
    inputs = {
        "x": rng.standard_normal((B, NF), dtype=np.float32),
        "conv_w": np.ones((3, 3), dtype=np.float32),
        "w1": (rng.standard_normal((676, H1)) * 0.04).astype(np.float32),
        "b1": np.zeros(H1, dtype=np.float32),
        "w2": (rng.standard_normal((H1, HO)) * 0.1).astype(np.float32),
        "b2": np.zeros(HO, dtype=np.float32),
        "w3": (rng.standard_normal((H1, HO))[:HO] * 0.3).astype(np.float32),
        "b3": np.zeros(HO, dtype=np.float32),
    }
    out = kernel(**inputs)
    print(out.shape, out.dtype)


# revision 12
# speedup vs baseline: 1.0373x; 1.0373x over previous
"""Trainium2 Bass kernel for DigitConvolutionalModel.

Model: x[B,784] -> reshape 28x28 -> 3x3 valid conv (weights conv_w) ->
[B,676] -> Linear(676,100)+relu -> Linear(100,10)+relu -> Linear(10,10).

The conv is linear, so it folds into the first Linear: W1f = C @ w1 where
C[784,676] is the conv unfold matrix. The whole model becomes a 3-layer MLP
784 -> 100 -> 10 -> 10 with relu between layers.

Sharding: pure data parallel, batch split across 8 cores (8192 rows each).

Precision: matmuls in bf16, accumulation in fp32 PSUM, biases + output fp32.

On-chip layout: activations stay feature-major ([features, batch]) end to
end; weights in natural [in,out] layout are the stationary operand.

v2 structure (from trace analysis of v1 @ 63.9us):
- x loads are granule-sized ([1,1,2,2,2,2,2,2,2] supertiles per dma_start,
  12KB/partition for the pair granules) on the sync queue, every granule in
  its own SBUF buffer (96KB total, no reuse waits on the load queue). One
  DMA sem per granule: PE matmuls carrying sem waits cost +88ns each
  (SW-decode path), so fewer, bigger transfers cut the PE wait tax.
- weights ride the scalar (ACT) HWDGE queue: that engine finishes its
  preamble ~1.5us before sync, so the blob lands before the first x tile.
- per-pair [*, 2, TN] PSUM tiles spanning 2 banks: one ACT activation per
  layer per pair (instead of per supertile) halves ACT instructions and PE
  cross-engine waits. PSUM budget: L1 2x2 + L2 2 + L3 2 = 8 banks.
- b3 is applied per-partition by the DVE scalar_tensor_tensor (add, +0),
  dropping the broadcast b3 image from the blob (447KB -> 186KB).
- short warmup (9 x 256-col dummy matmuls) bridges PE boot (~6.8us) to
  first-data (~10us) so the HAM clock is warm when real matmuls start.
- last pair processed as two split supertile chains (ACT half / DVE half in
  parallel) to shorten the serial L1->L2->L3->store tail.
"""

import numpy as np
import ml_dtypes

import concourse.bacc as bacc
import concourse.tile as tile
from concourse.tile import add_dep_helper
from concourse import mybir
from concourse.bass_utils import run_bass_kernel_spmd

N_CORES = 8
B = 65536
BC = B // N_CORES  # 8192 rows per core
TN = 512           # batch columns per supertile
NT = BC // TN      # 16 supertiles per core
NKC = 6            # full 128-feature chunks (0..767)
KT = 16            # tail features (768..783)
NF = 784
H1 = 100
HO = 10
F32 = mybir.dt.float32
BF16 = mybir.dt.bfloat16
NP_BF16 = ml_dtypes.bfloat16

# loads: (supertile, first chunk, n chunks). Uniform 2-supertile loads:
# 12KB/partition descriptors reach line rate from the first transfer
# (smaller descriptors measured latency-bound: 3KB -> ~100-200GB/s,
# 6KB -> ~280, 12KB -> ~375-424).
LOADS = [(2 * p, 0, 12) for p in range(NT // 2)]
NPAIR = NT // 2

# packed weight blob column layout (bf16 columns)
_C_W1M = 0                      # [128, 600]  w1m chunks
_C_W1T = 600                    # [16, 100]   w1t
_C_W2 = 700                     # [100, 10]   w2
_C_W3 = 710                     # [10, 10]    w3
_C_B1 = 720                     # [100, 2]    b1 as f32 byte-pairs
_C_B2 = 722                     # [10, 2]     b2
_C_B3 = 724                     # [10, 2]     b3
WBW = 726

N_WARMUP = 18
WUN = 256  # warmup matmul free dim


def _build_nc():
    nc = bacc.Bacc(None, target_bir_lowering=False)

    # feature-major, partition-major-first so any run of supertiles is
    # contiguous per partition: xt[p, t, k, n]
    xt = nc.dram_tensor("xt", [128, NT, NKC, TN], BF16, kind="ExternalInput")
    xt_tail = nc.dram_tensor("xt_tail", [KT, NT, TN], BF16, kind="ExternalInput")
    wblob = nc.dram_tensor("wblob", [128, WBW], BF16, kind="ExternalInput")
    yt = nc.dram_tensor("yt", [HO, BC], F32, kind="ExternalOutput")

    relu = mybir.ActivationFunctionType.Relu
    copy_fn = mybir.ActivationFunctionType.Identity

    with tile.TileContext(nc) as tc:
        with (
            tc.tile_pool(name="const", bufs=1) as cpool,
            tc.tile_pool(name="io", bufs=1) as iopool,
            tc.tile_pool(name="act", bufs=3) as apool,
            tc.tile_pool(name="ps1", bufs=2, space="PSUM") as ps1,
            tc.tile_pool(name="ps2", bufs=1, space="PSUM") as ps2,
            tc.tile_pool(name="ps3", bufs=1, space="PSUM") as ps3,
        ):
            # weights on the scalar HWDGE queue (boots earliest)
            wb_s = cpool.tile([128, WBW], BF16, tag="wb")
            nc.scalar.dma_start(wb_s[:], wblob[:])
            # tails on the gpsimd (SWDGE) queue
            xtl_s = cpool.tile([KT, NT, TN], BF16, tag="xtl")
            nc.gpsimd.dma_start(xtl_s[:], xt_tail[:])
            # loads, all issued upfront on the sync queue, each into its own
            # buffer (no reuse waits on the load queue)
            chunk_ap = {}
            for li, (t0, k0, nk) in enumerate(LOADS):
                t_ = iopool.tile([128, nk, TN], BF16, tag=f"x{li}")
                if nk > NKC:
                    s = nk // NKC
                    nc.sync.dma_start(t_[:], xt[:, t0:t0 + s])
                    for j in range(s):
                        for k in range(NKC):
                            chunk_ap[(t0 + j, k)] = t_[:, j * NKC + k, :]
                else:
                    nc.sync.dma_start(t_[:], xt[:, t0, k0:k0 + nk])
                    for k in range(nk):
                        chunk_ap[(t0, k0 + k)] = t_[:, k, :]

            w1t_ap = wb_s[0:KT, _C_W1T:_C_W1T + H1]
            w2_ap = wb_s[0:H1, _C_W2:_C_W2 + HO]
            w3_ap = wb_s[0:HO, _C_W3:_C_W3 + HO]
            b1_ap = wb_s[0:H1, _C_B1:_C_B1 + 2].bitcast(F32)
            b2_ap = wb_s[0:HO, _C_B2:_C_B2 + 2].bitcast(F32)
            b3_ap = wb_s[0:HO, _C_B3:_C_B3 + 2].bitcast(F32)

            # All matmuls chained with same-engine ordering deps so the PE
            # executes them in emission order (required for ldweights=False
            # weight reuse from the previous matmul).
            prev_mm = [None]

            def mm(out_ap, lhsT_ap, rhs_ap, start, stop, ldw=True):
                m = nc.tensor.matmul(out_ap, lhsT_ap, rhs_ap,
                                     start=start, stop=stop)
                if not ldw:
                    m.ins.ldweights = False
                if prev_mm[0] is not None:
                    add_dep_helper(m.ins, prev_mm[0], sync=False,
                                   reason="pe program order")
                prev_mm[0] = m.ins
                return m

            # Warmup: dummy matmuls bridge the PE-boot -> first-data window
            # so the HAM clock is warm for the real stream. They multiply
            # garbage (wsc is memset AFTER emission: WAR, not RAW, so they
            # start at the engines-go barrier).
            wsc = cpool.tile([128, 2 * TN], BF16, tag="wsc")
            wp = ps1.tile([H1, 2, TN], F32, tag="p1")
            mm(wp[:, 0, 0:WUN], wsc[:, 0:H1], wsc[:, 0:WUN],
               start=True, stop=True)
            for _ in range(N_WARMUP - 1):
                mm(wp[:, 0, 0:WUN], wsc[:, 0:H1], wsc[:, 0:WUN],
                   start=True, stop=True, ldw=False)
            nc.vector.memset(wsc[:], 0.0)

            def xap(t, k):
                return chunk_ap[(t, k)]

            h1s: dict[int, object] = {}
            h2s: dict[int, object] = {}

            def emit_l1(p):
                a, b = 2 * p, 2 * p + 1
                last = p == NPAIR - 1
                p1 = ps1.tile([H1, 2, TN], F32, tag="p1")
                for k in range(NKC):
                    mm(p1[:, 0, :], wb_s[:, k * H1:(k + 1) * H1],
                       xap(a, k), start=(k == 0), stop=False)
                    mm(p1[:, 1, :], wb_s[:, k * H1:(k + 1) * H1],
                       xap(b, k), start=(k == 0), stop=False, ldw=False)
                mm(p1[:, 0, :], w1t_ap, xtl_s[:, a, :],
                   start=False, stop=True)
                mm(p1[:, 1, :], w1t_ap, xtl_s[:, b, :],
                   start=False, stop=True, ldw=False)
                h1 = apool.tile([H1, 2, TN], BF16, tag="h1")
                if last:
                    # last pair: two parallel half-chains (ACT / DVE) to
                    # shorten the serial drain after the final L1 matmul
                    nc.scalar.activation(h1[:, 0, :], p1[:, 0, :], relu,
                                         bias=b1_ap)
                    nc.vector.scalar_tensor_tensor(
                        h1[:, 1, :], p1[:, 1, :], b1_ap, wsc[0:H1, 0:TN],
                        op0=mybir.AluOpType.add, op1=mybir.AluOpType.max)
                else:
                    nc.scalar.activation(h1[:], p1[:], relu, bias=b1_ap)
                h1s[p] = h1

            def emit_l2(p):
                h1 = h1s.pop(p)
                p2 = ps2.tile([HO, 2, TN], F32, tag="p2")
                mm(p2[:, 0, :], w2_ap, h1[:, 0, :], start=True, stop=True)
                mm(p2[:, 1, :], w2_ap, h1[:, 1, :], start=True, stop=True,
                   ldw=False)
                h2 = apool.tile([HO, 2, TN], BF16, tag="h2")
                if p == NPAIR - 1:
                    nc.scalar.activation(h2[:, 0, :], p2[:, 0, :], relu,
                                         bias=b2_ap)
                    nc.vector.scalar_tensor_tensor(
                        h2[:, 1, :], p2[:, 1, :], b2_ap, wsc[0:HO, 0:TN],
                        op0=mybir.AluOpType.add, op1=mybir.AluOpType.max)
                else:
                    nc.scalar.activation(h2[:], p2[:], relu, bias=b2_ap)
                h2s[p] = h2

            def emit_l3(p):
                h2 = h2s.pop(p)
                p3 = ps3.tile([HO, 2, TN], F32, tag="p3")
                mm(p3[:, 0, :], w3_ap, h2[:, 0, :], start=True, stop=True)
                mm(p3[:, 1, :], w3_ap, h2[:, 1, :], start=True, stop=True,
                   ldw=False)
                ot = apool.tile([HO, 2, TN], F32, tag="ot")
                dst = yt[:, 2 * p * TN:(2 * p + 2) * TN]
                if p == NPAIR - 1:
                    # split halves: a finishes on ACT (Identity+bias), b on
                    # DVE; stores issue as each half lands
                    nc.scalar.activation(ot[:, 0, :], p3[:, 0, :], copy_fn,
                                         bias=b3_ap)
                    nc.sync.dma_start(yt[:, 2 * p * TN:(2 * p + 1) * TN],
                                      ot[:, 0, :])
                    nc.vector.scalar_tensor_tensor(
                        ot[:, 1, :], p3[:, 1, :], b3_ap, wsc[0:HO, 0:TN],
                        op0=mybir.AluOpType.add, op1=mybir.AluOpType.add)
                    nc.sync.dma_start(yt[:, (2 * p + 1) * TN:(2 * p + 2) * TN],
                                      ot[:, 1, :])
                else:
                    nc.vector.scalar_tensor_tensor(
                        ot[:], p3[:], b3_ap, wsc[0:HO, :],
                        op0=mybir.AluOpType.add, op1=mybir.AluOpType.add)
                    # stores ride gpsimd so they never queue behind loads
                    # or block the sync engine mid-body
                    if p == NPAIR - 2:
                        nc.sync.dma_start(dst, ot[:])
                    else:
                        nc.gpsimd.dma_start(dst, ot[:])

            # 3-stage software pipeline: L1(p), L2(p-1), L3(p-2)
            for p in range(NPAIR + 2):
                if p < NPAIR:
                    emit_l1(p)
                if 1 <= p <= NPAIR:
                    emit_l2(p - 1)
                if p >= 2:
                    emit_l3(p - 2)

    nc.compile()
    return nc


def _fold_conv_into_w1(conv_w: np.ndarray, w1: np.ndarray) -> np.ndarray:
    """W1f[784,100] such that x @ W1f == conv(x).reshape(B,676) @ w1."""
    c = np.zeros((NF, 26 * 26), dtype=np.float64)
    for di in range(3):
        for dj in range(3):
            ii, jj = np.meshgrid(np.arange(26), np.arange(26), indexing="ij")
            src = (ii + di) * 28 + (jj + dj)
            dst = ii * 26 + jj
            c[src.ravel(), dst.ravel()] += np.float64(conv_w[di, dj])
    return (c @ w1.astype(np.float64)).astype(np.float32)


def _prep_in_maps(x, conv_w, w1, b1, w2, b2, w3, b3):
    x = np.asarray(x, dtype=np.float32)
    conv_w = np.asarray(conv_w, dtype=np.float32)
    w1 = np.asarray(w1, dtype=np.float32)
    b1 = np.asarray(b1, dtype=np.float32)
    w2 = np.asarray(w2, dtype=np.float32)
    b2 = np.asarray(b2, dtype=np.float32)
    w3 = np.asarray(w3, dtype=np.float32)
    b3 = np.asarray(b3, dtype=np.float32)

    w1f = _fold_conv_into_w1(conv_w, w1)  # [784, 100]
    # main chunks: feature f = k*128 + p -> [128, 600]
    w1m = np.ascontiguousarray(
        w1f[: 128 * NKC].reshape(NKC, 128, H1).transpose(1, 0, 2)
    ).astype(NP_BF16).reshape(128, NKC * H1)
    w1t = w1f[128 * NKC:].astype(NP_BF16)  # [16, 100]

    blob = np.zeros((128, WBW), np.uint16)
    blob[:, _C_W1M:_C_W1M + NKC * H1] = w1m.view(np.uint16)
    blob[0:KT, _C_W1T:_C_W1T + H1] = w1t.view(np.uint16)
    blob[0:H1, _C_W2:_C_W2 + HO] = w2.astype(NP_BF16).view(np.uint16)
    blob[0:HO, _C_W3:_C_W3 + HO] = w3.astype(NP_BF16).view(np.uint16)
    blob[0:H1, _C_B1:_C_B1 + 2] = b1.reshape(H1, 1).view(np.uint16)
    blob[0:HO, _C_B2:_C_B2 + 2] = b2.reshape(HO, 1).view(np.uint16)
    blob[0:HO, _C_B3:_C_B3 + 2] = b3.reshape(HO, 1).view(np.uint16)
    shared = {"wblob": blob.view(NP_BF16)}

    xb = x.astype(NP_BF16)  # cast once, full batch
    in_maps = []
    for core in range(N_CORES):
        xc = xb[core * BC:(core + 1) * BC]  # [8192, 784] bf16
        xct = xc.reshape(NT, TN, NF).transpose(0, 2, 1)  # [NT, NF, TN]
        # [128, NT, NKC, TN]: partition-major so granules are contiguous
        xt_main = np.ascontiguousarray(
            xct[:, : 128 * NKC].reshape(NT, NKC, 128, TN).transpose(2, 0, 1, 3)
        )
        xt_tail = np.ascontiguousarray(
            xct[:, 128 * NKC:].transpose(1, 0, 2)
        )  # [KT, NT, TN]
        in_maps.append({"xt": xt_main, "xt_tail": xt_tail, **shared})
    return in_maps


_NC = None


def _get_nc():
    global _NC
    if _NC is None:
        _NC = _build_nc()
    return _NC


def kernel(x, conv_w, w1, b1, w2, b2, w3, b3):
    in_maps = _prep_in_maps(x, conv_w, w1, b1, w2, b2, w3, b3)
    nc = _get_nc()
    res = run_bass_kernel_spmd(nc, in_maps, core_ids=list(range(N_CORES)))
    out = np.empty((B, HO), dtype=np.float32)
    for i in range(N_CORES):
        out[i * BC:(i + 1) * BC] = res.results[i]["yt"].T
    return out


if __name__ == "__main__":
    rng = np.random.default_rng(0)
    inputs = {
        "x": rng.standard_normal((B, NF), dtype=np.float32),
        "conv_w": np.ones((3, 3), dtype=np.float32),
        "w1": (rng.standard_normal((676, H1)) * 0.04).astype(np.float32),
        "b1": np.zeros(H1, dtype=np.float32),
        "w2": (rng.standard_normal((H1, HO)) * 0.1).astype(np.float32),
        "b2": np.zeros(HO, dtype=np.float32),
        "w3": (rng.standard_normal((H1, HO))[:HO] * 0.3).astype(np.float32),
        "b3": np.zeros(HO, dtype=np.float32),
    }
    out = kernel(**inputs)
    print(out.shape, out.dtype)
